# revision 7
# baseline (speedup 1.0000x reference)
"""MoE MLP kernel for Trainium2 (8 NeuronCores, Bass/Tile).

Problem: y = concat(h @ W2 + b2, h @ We[idx_b] + be[idx_b]) where
h = gelu(x @ W1 + b1), x: [16, 2048, 1024] f32, W1: [1024, 4096],
W2: [4096, 768], We: [8, 4096, 256], idx: [16] in [0, 8).

Sharding: data-parallel over batch B=16 -> 2 batch elements per core.
The expert selection is resolved on the host (indices are host-visible),
so each core runs one dense GEMM pipeline with its own gathered expert
weights - perfectly balanced, no collectives.

Per-core pipeline (4096 tokens, token tiles of TT=1024):
  Phase A: hT[hid, tok] = gelu(W1.T @ xT + b1) - x is pre-transposed on
    host so the contraction dim (IN) lies on SBUF partitions for both
    operands; output hT comes out HID-major, which is exactly the lhsT
    layout phase B needs. 8 K-chunks accumulate in PSUM; ScalarE applies
    bias+gelu (erf-exact on TRN2) while moving PSUM->SBUF.
  Phase B: out[tok, col] = hT.T @ Wcat + bcat with Wcat = [W2 | We_sel]
    [4096, 1024]: stationary = hT chunks, moving = Wcat column slabs.
    Column halves (h) are the outer loop so the 8 token-groups x 1
    col-half accumulation groups exactly fill the 8 PSUM banks while Wcat
    streams from HBM once per token tile. VectorE fuses bias-add with the
    PSUM->SBUF copy; outputs land token-major -> straight DMA to DRAM.

All matmuls run in float32r (TF32-like, 1 cycle/row vs 4 for fp32;
measured ~2e-4 rel err per K=128 chunk on HW).
"""

import os
import sys

sys.path.insert(0, "/opt/trn_rl_repo")

import numpy as np

import concourse.bass as bass  # noqa: F401  (engine namespaces live on nc)
import concourse.mybir as mybir
import concourse.tile as tile
from concourse import bacc, bass2jax

# Problem dims (hardcoded per contract)
IN, HID, OUT, PART, E = 1024, 4096, 1024, 256, 8
B, N_TOK = 16, 2048
NCORES = 8
BPC = B // NCORES            # batch elements per core
T_CORE = BPC * N_TOK         # tokens per core
TT = 1024                    # token tile
NT = T_CORE // TT            # token tiles per core
TPB = N_TOK // TT            # token tiles per batch element
KC = IN // 128               # fc1 contraction chunks
MC = HID // 128              # hidden chunks
F32 = mybir.dt.float32
F32R = mybir.dt.float32r

_CACHE = {}
LAST_RESULTS = None  # BassKernelResults of the most recent run (for test.py)


def _build_nc():
    nc = bacc.Bacc(None, target_bir_lowering=False, debug=False)

    xt_d = nc.dram_tensor("xt", [IN, T_CORE], F32R, kind="ExternalInput")
    w1_d = nc.dram_tensor("w1", [MC, 128, IN], F32R, kind="ExternalInput")
    b1_d = nc.dram_tensor("b1r", [128, MC], F32, kind="ExternalInput")
    wb_d = nc.dram_tensor("wb", [BPC, 2, MC, 128, 512], F32R, kind="ExternalInput")
    bb_d = nc.dram_tensor("bb", [BPC, 128, OUT], F32, kind="ExternalInput")
    out_d = nc.dram_tensor("out", [T_CORE, OUT], F32, kind="ExternalOutput")

    with tile.TileContext(nc) as tc:
        with (
            tc.tile_pool(name="const", bufs=1) as cpool,
            tc.tile_pool(name="h", bufs=MC) as hpool,
            tc.tile_pool(name="x", bufs=KC) as xpool,
            tc.tile_pool(name="w1", bufs=3) as w1pool,
            tc.tile_pool(name="wb", bufs=4) as wbpool,
            tc.tile_pool(name="o", bufs=4) as opool,
            tc.tile_pool(name="ps", bufs=8, space="PSUM") as pspool,
        ):
            b1_sb = cpool.tile([128, MC], F32, tag="b1")
            nc.sync.dma_start(b1_sb[:], b1_d[:])
            bb_sb = []
            for j in range(BPC):
                t_ = cpool.tile([128, OUT], F32, tag=f"bb{j}")
                nc.sync.dma_start(t_[:], bb_d[j])
                bb_sb.append(t_)

            for t in range(NT):
                j = t // TPB  # batch element of this token tile

                # ---- Phase A: hT = gelu(W1.T @ xT + b1) for this tile ----
                xts = []
                for k in range(KC):
                    xk = xpool.tile([128, TT], F32R, tag="x")
                    nc.sync.dma_start(
                        xk[:], xt_d[k * 128 : (k + 1) * 128, t * TT : (t + 1) * TT]
                    )
                    xts.append(xk)

                hts = []
                for m in range(MC):
                    w1m = w1pool.tile([128, IN], F32R, tag="w1")
                    nc.sync.dma_start(w1m[:], w1_d[m])
                    hm = hpool.tile([128, TT], F32R, tag="h")
                    for s in range(TT // 512):
                        psa = pspool.tile([128, 512], F32, tag="ps")
                        for k in range(KC):
                            nc.tensor.matmul(
                                psa[:],
                                w1m[:, k * 128 : (k + 1) * 128],
                                xts[k][:, s * 512 : (s + 1) * 512],
                                start=(k == 0),
                                stop=(k == KC - 1),
                            )
                        nc.scalar.activation(
                            hm[:, s * 512 : (s + 1) * 512],
                            psa[:],
                            mybir.ActivationFunctionType.Gelu,
                            bias=b1_sb[:, m : m + 1],
                        )
                    hts.append(hm)

                # ---- Phase B: out = hT.T @ [W2 | We_sel] + bias ----
                for h in range(2):
                    pso = [
                        pspool.tile([128, 512], F32, tag="ps", name=f"pso{g}")
                        for g in range(8)
                    ]
                    for m in range(MC):
                        wbm = wbpool.tile([128, 512], F32R, tag="wb")
                        nc.sync.dma_start(wbm[:], wb_d[j, h, m])
                        for g in range(8):
                            nc.tensor.matmul(
                                pso[g][:],
                                hts[m][:, g * 128 : (g + 1) * 128],
                                wbm[:],
                                start=(m == 0),
                                stop=(m == MC - 1),
                            )
                    for g in range(8):
                        ob = opool.tile([128, 512], F32, tag="o")
                        nc.vector.tensor_add(
                            ob[:], pso[g][:], bb_sb[j][:, h * 512 : (h + 1) * 512]
                        )
                        nc.sync.dma_start(
                            out_d[
                                t * TT + g * 128 : t * TT + (g + 1) * 128,
                                h * 512 : (h + 1) * 512,
                            ],
                            ob[:],
                        )

    nc.compile()
    return nc


def _make_runner(nc):
    """Cached executor mirroring bass2jax.run_bass_via_pjrt's multi-core
    path, but reusable: the jitted body + device-resident inputs persist
    across calls so repeat executions measure device time, not transfers."""
    import jax
    from jax.experimental.shard_map import shard_map
    from jax.sharding import Mesh, NamedSharding, PartitionSpec

    bass2jax.install_neuronx_cc_hook()

    partition_name = (
        nc.partition_id_tensor.name if nc.partition_id_tensor else None
    )
    in_names, out_names, out_avals, zero_outs = [], [], [], []
    for alloc in nc.m.functions[0].allocations:
        if not isinstance(alloc, mybir.MemoryLocationSet):
            continue
        name = alloc.memorylocations[0].name
        if alloc.kind == "ExternalInput":
            if name != partition_name:
                in_names.append(name)
        elif alloc.kind == "ExternalOutput":
            out_avals.append(
                jax.core.ShapedArray(alloc.tensor_shape, mybir.dt.np(alloc.dtype))
            )
            zero_outs.append(
                np.zeros(alloc.tensor_shape, dtype=mybir.dt.np(alloc.dtype))
            )
            out_names.append(name)

    n_params = len(in_names)
    all_names = in_names + out_names
    if partition_name is not None:
        all_names = all_names + [partition_name]

    def _body(*args):
        operands = list(args)
        if partition_name is not None:
            operands.append(bass2jax.partition_id_tensor())
        outs = bass2jax._bass_exec_p.bind(
            *operands,
            out_avals=tuple(out_avals),
            in_names=tuple(all_names),
            out_names=tuple(out_names),
            lowering_input_output_aliases=(),
            sim_require_finite=True,
            sim_require_nnan=True,
            nc=nc,
        )
        return tuple(outs)

    devices = jax.devices()[:NCORES]
    mesh = Mesh(np.asarray(devices), ("core",))
    spec = NamedSharding(mesh, PartitionSpec("core"))
    n_outs = len(out_names)
    sharded = jax.jit(
        shard_map(
            _body,
            mesh=mesh,
            in_specs=(PartitionSpec("core"),) * (n_params + n_outs),
            out_specs=(PartitionSpec("core"),) * n_outs,
            check_rep=False,
        ),
        donate_argnums=tuple(range(n_params, n_params + n_outs)),
        keep_unused=True,
    )

    def put_inputs(in_maps):
        concat = [
            np.concatenate([np.asarray(m[name]) for m in in_maps], axis=0)
            for name in in_names
        ]
        return [jax.device_put(a, spec) for a in concat]

    def put_zeros():
        return [
            jax.device_put(
                np.zeros((NCORES * z.shape[0], *z.shape[1:]), z.dtype), spec
            )
            for z in zero_outs
        ]

    def run(dev_inputs, dev_zeros):
        out_arrs = sharded(*dev_inputs, *dev_zeros)
        return [
            {
                name: np.asarray(out_arrs[i]).reshape(
                    NCORES, *out_avals[i].shape
                )[c]
                for i, name in enumerate(out_names)
            }
            for c in range(NCORES)
        ]

    return {
        "run": run,
        "put_inputs": put_inputs,
        "put_zeros": put_zeros,
        "sharded": sharded,
        "out_names": out_names,
    }


def get_runner():
    if "nc" not in _CACHE:
        _CACHE["nc"] = _build_nc()
    if "runner" not in _CACHE:
        _CACHE["runner"] = _make_runner(_CACHE["nc"])
    return _CACHE["runner"]


def make_in_maps(x, indices, W1, b1, W2, b2, We, be):
    # Replicated weights, rearranged so every DMA is a contiguous slab:
    # w1r[m, p, k*128+q] = W1[k*128+p, m*128+q]
    w1r = np.ascontiguousarray(
        W1.reshape(KC, 128, MC, 128).transpose(2, 1, 0, 3).reshape(MC, 128, IN)
    )
    b1r = np.ascontiguousarray(b1.reshape(MC, 128).T)

    in_maps = []
    for c in range(NCORES):
        xt = np.ascontiguousarray(x[c * BPC : (c + 1) * BPC].reshape(T_CORE, IN).T)
        wb = np.empty((BPC, 2, MC, 128, 512), dtype=np.float32)
        bb = np.empty((BPC, 128, OUT), dtype=np.float32)
        for jj in range(BPC):
            e = int(indices[c * BPC + jj])
            wcat = np.concatenate([W2, We[e]], axis=1)  # [HID, OUT]
            wb[jj] = wcat.reshape(MC, 128, 2, 512).transpose(2, 0, 1, 3)
            bb[jj] = np.concatenate([b2, be[e]])[None, :]
        in_maps.append({"xt": xt, "w1": w1r, "b1r": b1r, "wb": wb, "bb": bb})
    return in_maps


def kernel(x, indices, W1, b1, W2, b2, We, be):
    global LAST_RESULTS
    x = np.ascontiguousarray(np.asarray(x, dtype=np.float32))
    indices = np.asarray(indices).astype(np.int64)
    W1 = np.asarray(W1, dtype=np.float32)
    b1 = np.asarray(b1, dtype=np.float32)
    W2 = np.asarray(W2, dtype=np.float32)
    b2 = np.asarray(b2, dtype=np.float32)
    We = np.asarray(We, dtype=np.float32)
    be = np.asarray(be, dtype=np.float32)

    runner = get_runner()
    in_maps = make_in_maps(x, indices, W1, b1, W2, b2, We, be)
    dev_in = runner["put_inputs"](in_maps)
    results = runner["run"](dev_in, runner["put_zeros"]())
    LAST_RESULTS = results

    out = np.empty((B, N_TOK, OUT), dtype=np.float32)
    for c in range(NCORES):
        out[c * BPC : (c + 1) * BPC] = results[c]["out"].reshape(BPC, N_TOK, OUT)
    return out


# revision 10
# speedup vs baseline: 3.4215x; 3.4215x over previous
"""MoE MLP kernel for Trainium2 (8 NeuronCores, Bass/Tile).

Problem: y = concat(h @ W2 + b2, h @ We[idx_b] + be[idx_b]) where
h = gelu(x @ W1 + b1), x: [16, 2048, 1024] f32, W1: [1024, 4096],
W2: [4096, 768], We: [8, 4096, 256], idx: [16] in [0, 8).

Sharding: data-parallel over batch B=16 -> 2 batch elements per core.
The expert selection is resolved on the host (indices are host-visible),
so each core runs one dense GEMM pipeline with its own gathered expert
weights - perfectly balanced, no collectives.

Per-core pipeline (4096 tokens, token tiles of TT=1024):
  Phase A: hT[hid, tok] = gelu(W1.T @ xT + b1) - x is pre-transposed on
    host so the contraction dim (IN) lies on SBUF partitions for both
    operands; output hT comes out HID-major, which is exactly the lhsT
    layout phase B needs. 8 K-chunks accumulate in PSUM; ScalarE applies
    bias+gelu (erf-exact on TRN2) while moving PSUM->SBUF.
  Phase B: out[tok, col] = hT.T @ Wcat + bcat with Wcat = [W2 | We_sel]
    [4096, 1024]: stationary = hT chunks, moving = Wcat column slabs.
    Column halves (h) are the outer loop so the 8 token-groups x 1
    col-half accumulation groups exactly fill the 8 PSUM banks while Wcat
    streams from HBM once per token tile. VectorE fuses bias-add with the
    PSUM->SBUF copy; outputs land token-major -> straight DMA to DRAM.

All matmuls run in float32r (TF32-like, 1 cycle/row vs 4 for fp32;
measured ~2e-4 rel err per K=128 chunk on HW).
"""

import sys

sys.path.insert(0, "/opt/trn_rl_repo")

import numpy as np

import concourse.bass as bass  # noqa: F401  (engine namespaces live on nc)
import concourse.mybir as mybir
import concourse.tile as tile
from concourse import bacc, bass2jax

# Problem dims (hardcoded per contract)
IN, HID, OUT, PART, E = 1024, 4096, 1024, 256, 8
B, N_TOK = 16, 2048
NCORES = 8
BPC = B // NCORES            # batch elements per core
T_CORE = BPC * N_TOK         # tokens per core
TT = 1024                    # token tile
NT = T_CORE // TT            # token tiles per core
TPB = N_TOK // TT            # token tiles per batch element
KC = IN // 128               # fc1 contraction chunks
MC = HID // 128              # hidden chunks
F32 = mybir.dt.float32
F32R = mybir.dt.float32r

_CACHE = {}


def _build_nc(reps=1):
    """reps>1 repeats the full computation in one NEFF (timing variant:
    device time scales with reps while per-call dispatch cost does not)."""
    nc = bacc.Bacc(None, target_bir_lowering=False, debug=False)

    xt_d = nc.dram_tensor("xt", [IN, T_CORE], F32R, kind="ExternalInput")
    w1_d = nc.dram_tensor("w1", [MC, 128, IN], F32R, kind="ExternalInput")
    b1_d = nc.dram_tensor("b1r", [128, MC], F32, kind="ExternalInput")
    wb_d = nc.dram_tensor("wb", [BPC, 2, MC, 128, 512], F32R, kind="ExternalInput")
    bb_d = nc.dram_tensor("bb", [BPC, 128, OUT], F32, kind="ExternalInput")
    out_d = nc.dram_tensor("out", [T_CORE, OUT], F32, kind="ExternalOutput")

    with tile.TileContext(nc) as tc:
        with (
            tc.tile_pool(name="const", bufs=1) as cpool,
            tc.tile_pool(name="h", bufs=MC) as hpool,
            tc.tile_pool(name="x", bufs=KC) as xpool,
            tc.tile_pool(name="w1", bufs=3) as w1pool,
            tc.tile_pool(name="wb", bufs=4) as wbpool,
            tc.tile_pool(name="o", bufs=4) as opool,
            tc.tile_pool(name="ps", bufs=8, space="PSUM") as pspool,
        ):
            b1_sb = cpool.tile([128, MC], F32, tag="b1")
            nc.sync.dma_start(b1_sb[:], b1_d[:])
            bb_sb = []
            for j in range(BPC):
                t_ = cpool.tile([128, OUT], F32, tag=f"bb{j}")
                nc.sync.dma_start(t_[:], bb_d[j])
                bb_sb.append(t_)

            for t in [tt for _ in range(reps) for tt in range(NT)]:
                j = t // TPB  # batch element of this token tile

                # ---- Phase A: hT = gelu(W1.T @ xT + b1) for this tile ----
                xts = []
                for k in range(KC):
                    xk = xpool.tile([128, TT], F32R, tag="x")
                    nc.sync.dma_start(
                        xk[:], xt_d[k * 128 : (k + 1) * 128, t * TT : (t + 1) * TT]
                    )
                    xts.append(xk)

                hts = []
                for m in range(MC):
                    w1m = w1pool.tile([128, IN], F32R, tag="w1")
                    nc.sync.dma_start(w1m[:], w1_d[m])
                    hm = hpool.tile([128, TT], F32R, tag="h")
                    for s in range(TT // 512):
                        psa = pspool.tile([128, 512], F32, tag="ps")
                        for k in range(KC):
                            nc.tensor.matmul(
                                psa[:],
                                w1m[:, k * 128 : (k + 1) * 128],
                                xts[k][:, s * 512 : (s + 1) * 512],
                                start=(k == 0),
                                stop=(k == KC - 1),
                            )
                        nc.scalar.activation(
                            hm[:, s * 512 : (s + 1) * 512],
                            psa[:],
                            mybir.ActivationFunctionType.Gelu,
                            bias=b1_sb[:, m : m + 1],
                        )
                    hts.append(hm)

                # ---- Phase B: out = hT.T @ [W2 | We_sel] + bias ----
                for h in range(2):
                    pso = [
                        pspool.tile([128, 512], F32, tag="ps", name=f"pso{g}")
                        for g in range(8)
                    ]
                    for m in range(MC):
                        wbm = wbpool.tile([128, 512], F32R, tag="wb")
                        nc.sync.dma_start(wbm[:], wb_d[j, h, m])
                        for g in range(8):
                            nc.tensor.matmul(
                                pso[g][:],
                                hts[m][:, g * 128 : (g + 1) * 128],
                                wbm[:],
                                start=(m == 0),
                                stop=(m == MC - 1),
                            )
                    for g in range(8):
                        ob = opool.tile([128, 512], F32, tag="o")
                        nc.vector.tensor_add(
                            ob[:], pso[g][:], bb_sb[j][:, h * 512 : (h + 1) * 512]
                        )
                        nc.sync.dma_start(
                            out_d[
                                t * TT + g * 128 : t * TT + (g + 1) * 128,
                                h * 512 : (h + 1) * 512,
                            ],
                            ob[:],
                        )

    nc.compile()
    return nc


def _make_runner(nc):
    """Cached executor mirroring bass2jax.run_bass_via_pjrt's multi-core
    path, but reusable: the jitted body + device-resident inputs persist
    across calls so repeat executions measure device time, not transfers."""
    import jax
    from jax.experimental.shard_map import shard_map
    from jax.sharding import Mesh, NamedSharding, PartitionSpec

    bass2jax.install_neuronx_cc_hook()

    partition_name = (
        nc.partition_id_tensor.name if nc.partition_id_tensor else None
    )
    in_names, out_names, out_avals, zero_outs = [], [], [], []
    for alloc in nc.m.functions[0].allocations:
        if not isinstance(alloc, mybir.MemoryLocationSet):
            continue
        name = alloc.memorylocations[0].name
        if alloc.kind == "ExternalInput":
            if name != partition_name:
                in_names.append(name)
        elif alloc.kind == "ExternalOutput":
            out_avals.append(
                jax.core.ShapedArray(alloc.tensor_shape, mybir.dt.np(alloc.dtype))
            )
            zero_outs.append(
                np.zeros(alloc.tensor_shape, dtype=mybir.dt.np(alloc.dtype))
            )
            out_names.append(name)

    n_params = len(in_names)
    all_names = in_names + out_names
    if partition_name is not None:
        all_names = all_names + [partition_name]

    def _body(*args):
        operands = list(args)
        if partition_name is not None:
            operands.append(bass2jax.partition_id_tensor())
        outs = bass2jax._bass_exec_p.bind(
            *operands,
            out_avals=tuple(out_avals),
            in_names=tuple(all_names),
            out_names=tuple(out_names),
            lowering_input_output_aliases=(),
            sim_require_finite=True,
            sim_require_nnan=True,
            nc=nc,
        )
        return tuple(outs)

    devices = jax.devices()[:NCORES]
    mesh = Mesh(np.asarray(devices), ("core",))
    spec = NamedSharding(mesh, PartitionSpec("core"))
    n_outs = len(out_names)
    sharded = jax.jit(
        shard_map(
            _body,
            mesh=mesh,
            in_specs=(PartitionSpec("core"),) * (n_params + n_outs),
            out_specs=(PartitionSpec("core"),) * n_outs,
            check_rep=False,
        ),
        donate_argnums=tuple(range(n_params, n_params + n_outs)),
        keep_unused=True,
    )

    def put_inputs(in_maps):
        concat = [
            np.concatenate([np.asarray(m[name]) for m in in_maps], axis=0)
            for name in in_names
        ]
        return [jax.device_put(a, spec) for a in concat]

    def put_zeros():
        return [
            jax.device_put(
                np.zeros((NCORES * z.shape[0], *z.shape[1:]), z.dtype), spec
            )
            for z in zero_outs
        ]

    def run(dev_inputs, dev_zeros):
        out_arrs = sharded(*dev_inputs, *dev_zeros)
        return [
            {
                name: np.asarray(out_arrs[i]).reshape(
                    NCORES, *out_avals[i].shape
                )[c]
                for i, name in enumerate(out_names)
            }
            for c in range(NCORES)
        ]

    return {
        "run": run,
        "put_inputs": put_inputs,
        "put_zeros": put_zeros,
        "sharded": sharded,
        "out_names": out_names,
    }


def get_runner():
    if "nc" not in _CACHE:
        _CACHE["nc"] = _build_nc()
    if "runner" not in _CACHE:
        _CACHE["runner"] = _make_runner(_CACHE["nc"])
    return _CACHE["runner"]


def make_in_maps(x, indices, W1, b1, W2, b2, We, be):
    # Replicated weights, rearranged so every DMA is a contiguous slab:
    # w1r[m, p, k*128+q] = W1[k*128+p, m*128+q]
    w1r = np.ascontiguousarray(
        W1.reshape(KC, 128, MC, 128).transpose(2, 1, 0, 3).reshape(MC, 128, IN)
    )
    b1r = np.ascontiguousarray(b1.reshape(MC, 128).T)

    in_maps = []
    for c in range(NCORES):
        xt = np.ascontiguousarray(x[c * BPC : (c + 1) * BPC].reshape(T_CORE, IN).T)
        wb = np.empty((BPC, 2, MC, 128, 512), dtype=np.float32)
        bb = np.empty((BPC, 128, OUT), dtype=np.float32)
        for jj in range(BPC):
            e = int(indices[c * BPC + jj])
            wcat = np.concatenate([W2, We[e]], axis=1)  # [HID, OUT]
            wb[jj] = wcat.reshape(MC, 128, 2, 512).transpose(2, 0, 1, 3)
            bb[jj] = np.concatenate([b2, be[e]])[None, :]
        in_maps.append({"xt": xt, "w1": w1r, "b1r": b1r, "wb": wb, "bb": bb})
    return in_maps


def kernel(x, indices, W1, b1, W2, b2, We, be):
    x = np.ascontiguousarray(np.asarray(x, dtype=np.float32))
    indices = np.asarray(indices).astype(np.int64)
    W1 = np.asarray(W1, dtype=np.float32)
    b1 = np.asarray(b1, dtype=np.float32)
    W2 = np.asarray(W2, dtype=np.float32)
    b2 = np.asarray(b2, dtype=np.float32)
    We = np.asarray(We, dtype=np.float32)
    be = np.asarray(be, dtype=np.float32)

    runner = get_runner()
    in_maps = make_in_maps(x, indices, W1, b1, W2, b2, We, be)
    dev_in = runner["put_inputs"](in_maps)
    results = runner["run"](dev_in, runner["put_zeros"]())

    out = np.empty((B, N_TOK, OUT), dtype=np.float32)
    for c in range(NCORES):
        out[c * BPC : (c + 1) * BPC] = results[c]["out"].reshape(BPC, N_TOK, OUT)
    return out


# revision 15
# speedup vs baseline: 3.7557x; 1.0977x over previous
"""MoE MLP kernel for Trainium2 (8 NeuronCores, Bass/Tile).

Problem: y = concat(h @ W2 + b2, h @ We[idx_b] + be[idx_b]) where
h = gelu(x @ W1 + b1), x: [16, 2048, 1024] f32, W1: [1024, 4096],
W2: [4096, 768], We: [8, 4096, 256], idx: [16] in [0, 8).

Sharding: data-parallel over batch B=16 -> 2 batch elements per core.
The expert selection is resolved on the host (indices are host-visible),
so each core runs one dense GEMM pipeline with its own gathered expert
weights - perfectly balanced, no collectives.

Per-core pipeline (4096 tokens, token tiles of TT=1024):
  Phase A: hT[hid, tok] = gelu(W1.T @ xT + b1) - x is pre-transposed on
    host so the contraction dim (IN) lies on SBUF partitions for both
    operands; output hT comes out HID-major, which is exactly the lhsT
    layout phase B needs. 8 K-chunks accumulate in PSUM; ScalarE applies
    bias+gelu (erf-exact on TRN2) while moving PSUM->SBUF.
  Phase B: out[tok, col] = hT.T @ Wcat + bcat with Wcat = [W2 | We_sel]
    [4096, 1024]: stationary = hT chunks, moving = Wcat column slabs.
    Token halves (u) are the outer loop: 4 token-groups x 2 col-halves
    fill the 8 PSUM banks exactly, and each stationary hT chunk feeds
    both column halves back-to-back (stationary reload amortized; the
    fp32r reload tax is ~22 ns/MM). VectorE fuses bias-add with the
    PSUM->SBUF copy; outputs land token-major -> straight DMA to DRAM.

All matmuls run in float32r (TF32-like, 1 cycle/row vs 4 for fp32;
measured ~2e-4 rel err per K=128 chunk on HW).
"""

import sys

sys.path.insert(0, "/opt/trn_rl_repo")

import numpy as np

import concourse.bass as bass  # noqa: F401  (engine namespaces live on nc)
import concourse.mybir as mybir
import concourse.tile as tile
from concourse import bacc, bass2jax

# Problem dims (hardcoded per contract)
IN, HID, OUT, PART, E = 1024, 4096, 1024, 256, 8
B, N_TOK = 16, 2048
NCORES = 8
BPC = B // NCORES            # batch elements per core
T_CORE = BPC * N_TOK         # tokens per core
TT = 1024                    # token tile
NT = T_CORE // TT            # token tiles per core
TPB = N_TOK // TT            # token tiles per batch element
KC = IN // 128               # fc1 contraction chunks
MC = HID // 128              # hidden chunks
F32 = mybir.dt.float32
F32R = mybir.dt.float32r

_CACHE = {}


def _build_nc(reps=1):
    """reps>1 repeats the full computation in one NEFF (timing variant:
    device time scales with reps while per-call dispatch cost does not)."""
    nc = bacc.Bacc(None, target_bir_lowering=False, debug=False)

    xt_d = nc.dram_tensor("xt", [IN, T_CORE], F32R, kind="ExternalInput")
    w1_d = nc.dram_tensor("w1", [MC, 128, IN], F32R, kind="ExternalInput")
    b1_d = nc.dram_tensor("b1r", [128, MC], F32, kind="ExternalInput")
    wb_d = nc.dram_tensor("wb", [BPC, MC, 128, OUT], F32R, kind="ExternalInput")
    bb_d = nc.dram_tensor("bb", [BPC, 128, OUT], F32, kind="ExternalInput")
    out_d = nc.dram_tensor("out", [T_CORE, OUT], F32, kind="ExternalOutput")

    with tile.TileContext(nc) as tc:
        with (
            tc.tile_pool(name="const", bufs=1) as cpool,
            tc.tile_pool(name="h", bufs=MC) as hpool,
            tc.tile_pool(name="x", bufs=KC) as xpool,
            tc.tile_pool(name="w1", bufs=3) as w1pool,
            tc.tile_pool(name="wb", bufs=4) as wbpool,
            tc.tile_pool(name="o", bufs=4) as opool,
            tc.tile_pool(name="ps", bufs=8, space="PSUM") as pspool,
        ):
            b1_sb = cpool.tile([128, MC], F32, tag="b1")
            nc.sync.dma_start(b1_sb[:], b1_d[:])
            bb_sb = []
            for j in range(BPC):
                t_ = cpool.tile([128, OUT], F32, tag=f"bb{j}")
                nc.sync.dma_start(t_[:], bb_d[j])
                bb_sb.append(t_)

            for t in [tt for _ in range(reps) for tt in range(NT)]:
                j = t // TPB  # batch element of this token tile

                # ---- Phase A: hT = gelu(W1.T @ xT + b1) for this tile ----
                xts = []
                for k in range(KC):
                    xk = xpool.tile([128, TT], F32R, tag="x")
                    nc.sync.dma_start(
                        xk[:], xt_d[k * 128 : (k + 1) * 128, t * TT : (t + 1) * TT]
                    )
                    xts.append(xk)

                hts = []
                for m in range(MC):
                    w1m = w1pool.tile([128, IN], F32R, tag="w1")
                    nc.sync.dma_start(w1m[:], w1_d[m])
                    hm = hpool.tile([128, TT], F32R, tag="h")
                    # k-outer / s-inner: each stationary W1 chunk feeds the
                    # two 512-token subtiles back-to-back (reload amortized)
                    psa = [
                        pspool.tile([128, 512], F32, tag="ps", name=f"psa{s}")
                        for s in range(TT // 512)
                    ]
                    for k in range(KC):
                        for s in range(TT // 512):
                            nc.tensor.matmul(
                                psa[s][:],
                                w1m[:, k * 128 : (k + 1) * 128],
                                xts[k][:, s * 512 : (s + 1) * 512],
                                start=(k == 0),
                                stop=(k == KC - 1),
                            )
                    for s in range(TT // 512):
                        nc.scalar.activation(
                            hm[:, s * 512 : (s + 1) * 512],
                            psa[s][:],
                            mybir.ActivationFunctionType.Gelu,
                            bias=b1_sb[:, m : m + 1],
                        )
                    hts.append(hm)

                # ---- Phase B: out = hT.T @ [W2 | We_sel] + bias ----
                # Token halves (u) outer; per m a full-width [128, 1024]
                # Wcat slab streams in and each stationary hT chunk (m, g)
                # feeds both column halves back-to-back (reload amortized).
                # 4 token-groups x 2 col-halves = 8 PSUM banks per half.
                for u in range(2):
                    pso = [
                        pspool.tile([128, 512], F32, tag="ps", name=f"pso{i}")
                        for i in range(8)
                    ]
                    for m in range(MC):
                        wbm = wbpool.tile([128, OUT], F32R, tag="wb")
                        nc.sync.dma_start(wbm[:], wb_d[j, m])
                        for g in range(4):
                            for h in range(2):
                                nc.tensor.matmul(
                                    pso[g * 2 + h][:],
                                    hts[m][
                                        :,
                                        u * 512 + g * 128 : u * 512 + (g + 1) * 128,
                                    ],
                                    wbm[:, h * 512 : (h + 1) * 512],
                                    start=(m == 0),
                                    stop=(m == MC - 1),
                                )
                    for g in range(4):
                        for h in range(2):
                            ob = opool.tile([128, 512], F32, tag="o")
                            nc.vector.tensor_add(
                                ob[:],
                                pso[g * 2 + h][:],
                                bb_sb[j][:, h * 512 : (h + 1) * 512],
                            )
                            row0 = t * TT + u * 512 + g * 128
                            nc.sync.dma_start(
                                out_d[row0 : row0 + 128, h * 512 : (h + 1) * 512],
                                ob[:],
                            )

    nc.compile()
    return nc


def _make_runner(nc):
    """Cached executor mirroring bass2jax.run_bass_via_pjrt's multi-core
    path, but reusable: the jitted body + device-resident inputs persist
    across calls so repeat executions measure device time, not transfers."""
    import jax
    from jax.experimental.shard_map import shard_map
    from jax.sharding import Mesh, NamedSharding, PartitionSpec

    bass2jax.install_neuronx_cc_hook()

    partition_name = (
        nc.partition_id_tensor.name if nc.partition_id_tensor else None
    )
    in_names, out_names, out_avals, zero_outs = [], [], [], []
    for alloc in nc.m.functions[0].allocations:
        if not isinstance(alloc, mybir.MemoryLocationSet):
            continue
        name = alloc.memorylocations[0].name
        if alloc.kind == "ExternalInput":
            if name != partition_name:
                in_names.append(name)
        elif alloc.kind == "ExternalOutput":
            out_avals.append(
                jax.core.ShapedArray(alloc.tensor_shape, mybir.dt.np(alloc.dtype))
            )
            zero_outs.append(
                np.zeros(alloc.tensor_shape, dtype=mybir.dt.np(alloc.dtype))
            )
            out_names.append(name)

    n_params = len(in_names)
    all_names = in_names + out_names
    if partition_name is not None:
        all_names = all_names + [partition_name]

    def _body(*args):
        operands = list(args)
        if partition_name is not None:
            operands.append(bass2jax.partition_id_tensor())
        outs = bass2jax._bass_exec_p.bind(
            *operands,
            out_avals=tuple(out_avals),
            in_names=tuple(all_names),
            out_names=tuple(out_names),
            lowering_input_output_aliases=(),
            sim_require_finite=True,
            sim_require_nnan=True,
            nc=nc,
        )
        return tuple(outs)

    devices = jax.devices()[:NCORES]
    mesh = Mesh(np.asarray(devices), ("core",))
    spec = NamedSharding(mesh, PartitionSpec("core"))
    n_outs = len(out_names)
    sharded = jax.jit(
        shard_map(
            _body,
            mesh=mesh,
            in_specs=(PartitionSpec("core"),) * (n_params + n_outs),
            out_specs=(PartitionSpec("core"),) * n_outs,
            check_rep=False,
        ),
        donate_argnums=tuple(range(n_params, n_params + n_outs)),
        keep_unused=True,
    )

    def put_inputs(in_maps):
        concat = [
            np.concatenate([np.asarray(m[name]) for m in in_maps], axis=0)
            for name in in_names
        ]
        return [jax.device_put(a, spec) for a in concat]

    def put_zeros():
        return [
            jax.device_put(
                np.zeros((NCORES * z.shape[0], *z.shape[1:]), z.dtype), spec
            )
            for z in zero_outs
        ]

    def run(dev_inputs, dev_zeros):
        out_arrs = sharded(*dev_inputs, *dev_zeros)
        return [
            {
                name: np.asarray(out_arrs[i]).reshape(
                    NCORES, *out_avals[i].shape
                )[c]
                for i, name in enumerate(out_names)
            }
            for c in range(NCORES)
        ]

    return {
        "run": run,
        "put_inputs": put_inputs,
        "put_zeros": put_zeros,
        "sharded": sharded,
        "out_names": out_names,
    }


def get_runner():
    if "nc" not in _CACHE:
        _CACHE["nc"] = _build_nc()
    if "runner" not in _CACHE:
        _CACHE["runner"] = _make_runner(_CACHE["nc"])
    return _CACHE["runner"]


def make_in_maps(x, indices, W1, b1, W2, b2, We, be):
    # Replicated weights, rearranged so every DMA is a contiguous slab:
    # w1r[m, p, k*128+q] = W1[k*128+p, m*128+q]
    w1r = np.ascontiguousarray(
        W1.reshape(KC, 128, MC, 128).transpose(2, 1, 0, 3).reshape(MC, 128, IN)
    )
    b1r = np.ascontiguousarray(b1.reshape(MC, 128).T)

    in_maps = []
    for c in range(NCORES):
        xt = np.ascontiguousarray(x[c * BPC : (c + 1) * BPC].reshape(T_CORE, IN).T)
        wb = np.empty((BPC, MC, 128, OUT), dtype=np.float32)
        bb = np.empty((BPC, 128, OUT), dtype=np.float32)
        for jj in range(BPC):
            e = int(indices[c * BPC + jj])
            wcat = np.concatenate([W2, We[e]], axis=1)  # [HID, OUT]
            wb[jj] = wcat.reshape(MC, 128, OUT)
            bb[jj] = np.concatenate([b2, be[e]])[None, :]
        in_maps.append({"xt": xt, "w1": w1r, "b1r": b1r, "wb": wb, "bb": bb})
    return in_maps


def kernel(x, indices, W1, b1, W2, b2, We, be):
    x = np.ascontiguousarray(np.asarray(x, dtype=np.float32))
    indices = np.asarray(indices).astype(np.int64)
    W1 = np.asarray(W1, dtype=np.float32)
    b1 = np.asarray(b1, dtype=np.float32)
    W2 = np.asarray(W2, dtype=np.float32)
    b2 = np.asarray(b2, dtype=np.float32)
    We = np.asarray(We, dtype=np.float32)
    be = np.asarray(be, dtype=np.float32)

    runner = get_runner()
    in_maps = make_in_maps(x, indices, W1, b1, W2, b2, We, be)
    dev_in = runner["put_inputs"](in_maps)
    results = runner["run"](dev_in, runner["put_zeros"]())

    out = np.empty((B, N_TOK, OUT), dtype=np.float32)
    for c in range(NCORES):
        out[c * BPC : (c + 1) * BPC] = results[c]["out"].reshape(BPC, N_TOK, OUT)
    return out
